# revision 28
# baseline (speedup 1.0000x reference)
"""Trainium2 Bass kernel for nn_AttentionBlock (B=4, D=128, T=4096, K=64, V=128).

Strategy (SPMD, 8 cores, identical program on every core — only input data
differs): core c processes the FULL batch element c % 4 (cores 4-7 duplicate
cores 0-3; the host uses outputs of cores 0-3). Per core:

  xT = minibatch[b]                      # [D=128, T] already transposed
  kT[k,t]  = Wk @ x^T  (+bk)             # PE: lhsT=WkT[128,64], rhs=xT
  qT[k,q]  = Wq @ x^T  (+bq)
  logitsT[t,q] = kT^T·qT   (PE, fp32r)   # [t-block 128, q-half 2048] tiles,
                                         # shrunk to the causal width
  e = exp(logitsT)  (ACT, accum_out -> per-t column sums)   # stored bf16
  gsum[t] = sum_q e[t,q]  (local — full q range on this core)
  v[t,:]  = (x@Wv^T + bv) / (8*gsum[t])  # folded softmax normalization
  readT[v,q] = sum_t v'[t,v] * e[t,q]    # PE accumulate over t-chunks
  out = xT + readT                       # [128, 4096] -> DRAM

Softmax here is over the QUERY axis (dim=1) per reference; no max-subtraction
is needed (logits are O(15), exp stays well inside fp32 range).
"""

import math
import numpy as np
from contextlib import ExitStack

B, D, T, K, V = 4, 128, 4096, 64, 128
NCORES = 8
HALF = T // 2  # 2048
NEG = -1.0e5  # additive mask; exp(x + NEG) == 0.0 in fp32

USE_F32R = True  # fp32r matmuls (full rate at N>=256); False -> plain fp32

_CACHE = {}


def _n_t(half):
    """Number of valid 128-t-blocks for query half `half` (q in [2048h, 2048h+2048))."""
    return 16 * (half + 1)


def _width(j, half):
    """Stored (causally shrunk) q-width of logits tile for t-block j, q-half `half`."""
    q0 = HALF * half
    return HALF - max(0, 128 * j - q0)


def build_program():
    import concourse.bass as bass
    import concourse.mybir as mybir
    import concourse.tile as tile
    from concourse import bacc

    f32 = mybir.dt.float32
    f32r = mybir.dt.float32r
    bf16 = mybir.dt.bfloat16

    def r(ap):
        return ap.bitcast(f32r) if USE_F32R else ap

    nc = bacc.Bacc(
        "TRN2",
        target_bir_lowering=False,
        debug=False,
        enable_asserts=False,
        num_devices=NCORES,
    )

    # ---- external I/O (per-core data differs, names/shapes identical) ----
    xT_d = nc.dram_tensor("xT", [D, T], f32r, kind="ExternalInput").ap()
    wkT_d = nc.dram_tensor("wkT", [D, K], f32r, kind="ExternalInput").ap()
    wqT_d = nc.dram_tensor("wqT", [D, K], f32r, kind="ExternalInput").ap()
    wvT_d = nc.dram_tensor("wvT", [D, V], f32r, kind="ExternalInput").ap()
    bk_d = nc.dram_tensor("bk", [K, 1], f32, kind="ExternalInput").ap()
    bq_d = nc.dram_tensor("bq", [K, 1], f32, kind="ExternalInput").ap()
    bvb_d = nc.dram_tensor("bvb", [128, V], f32, kind="ExternalInput").ap()
    mtri_d = nc.dram_tensor("mtri", [128, 128], f32, kind="ExternalInput").ap()
    out_d = nc.dram_tensor("out", [D, T], f32, kind="ExternalOutput").ap()

    # e-tile offsets (bf16 columns), uniform for every core
    e_off = {}
    off = 0
    for half in range(2):
        for j in range(_n_t(half)):
            w = _width(j, half)
            e_off[(half, j)] = (off, w)
            off += w
    E_COLS = off  # 67584

    with tile.TileContext(nc) as tc, ExitStack() as ctx:
        consts = ctx.enter_context(tc.tile_pool(name="consts", bufs=1))
        big = ctx.enter_context(tc.tile_pool(name="big", bufs=1))

        # ---- constants ----
        wk_sb = consts.tile([D, K], f32r, name="wk_sb")
        wq_sb = consts.tile([D, K], f32r, name="wq_sb")
        wv_sb = consts.tile([D, V], f32r, name="wv_sb")
        bk_sb = consts.tile([K, 1], f32, name="bk_sb")
        bq_sb = consts.tile([K, 1], f32, name="bq_sb")
        bvb_sb = consts.tile([128, V], f32, name="bvb_sb")
        mtri_sb = consts.tile([128, 128], f32, name="mtri_sb")
        nc.sync.dma_start(out=wk_sb, in_=wkT_d)
        nc.sync.dma_start(out=wq_sb, in_=wqT_d)
        nc.gpsimd.dma_start(out=wv_sb, in_=wvT_d)
        nc.gpsimd.dma_start(out=bk_sb, in_=bk_d)
        nc.gpsimd.dma_start(out=bq_sb, in_=bq_d)
        nc.gpsimd.dma_start(out=bvb_sb, in_=bvb_d)
        nc.gpsimd.dma_start(out=mtri_sb, in_=mtri_d)

        # ---- big persistent tiles ----
        xT_sb = big.tile([D, T], f32r, name="xT_sb")          # 16 KB/part
        kT_sb = big.tile([K, T], f32r, name="kT_sb")          # 16 KB/part
        qT_sb = big.tile([K, HALF], f32r, name="qT_sb")      # 8 KB/part
        vraw = big.tile([128, 32 * V], bf16, name="vraw")    # 8 KB/part
        e_all = big.tile([128, E_COLS], bf16, name="e_all")  # 132 KB/part
        sums = consts.tile([128, 128], f32, name="sums")     # 4 slots per t-block
        gsum = consts.tile([128, 32], f32, name="gsum")
        rcp = consts.tile([128, 32], f32, name="rcp")

        warm = consts.tile([1, 1], f32, name="warm")
        nc.vector.memset(warm, 0.0)
        nc.scalar.activation(warm, warm, mybir.ActivationFunctionType.Exp)
        for i in range(0, T, 512):  # chunked so phase A starts on first arrival
            nc.sync.dma_start(out=xT_sb[:, i : i + 512], in_=xT_d[:, i : i + 512])
        nc.vector.memset(sums, 0.0)

        # ---- phase A: kT projection for the first half of t only; the rest
        # is interleaved into half 0's loop (needed only by half 1)
        def kT_chunk(psA, i):
            c0, c1 = 512 * i, 512 * (i + 1)
            psk = psA.tile([K, 512], f32, name="psk", tag="psrq")
            nc.tensor.matmul(psk, wk_sb, xT_sb[:, c0:c1], start=True, stop=True)
            nc.vector.tensor_scalar_add(kT_sb[:, c0:c1], psk, bk_sb)


        # ---- phases B+C fused: logits+exp stream, with the read matmul
        # interleaved column-tile-major as per-t-block-group reciprocals
        # become available (so the PE read work hides under the ACT-bound
        # exp stream). psum: psB 4 banks + psQV 2 + psr 2 = 8.
        live_psr = {}

        def read_col_tile(owork, psR, h, qi, jlo, jhi):
            """Accumulate readT rows jlo..jhi for 512-col tile qi of half h;
            emit the output chunk when jhi is the tile's last t-block."""
            q0 = HALF * h
            jmax = 16 * h + 4 * qi + 3  # last contributing t-block
            if (h, qi) not in live_psr:
                live_psr[(h, qi)] = psR.tile([128, 512], f32, name="psr",
                                             tag="psrq")
            psr = live_psr[(h, qi)]
            for j in range(jlo, jhi + 1):
                eo, w = e_off[(h, j)]
                dd = HALF - w
                sc = max(512 * qi, dd)           # start col within the half
                sw = 512 * (qi + 1) - sc
                nc.tensor.matmul(
                    psr[:, sc - 512 * qi : sc - 512 * qi + sw],
                    vraw[:, V * j : V * (j + 1)],
                    e_all[:, eo + sc - dd : eo + sc - dd + sw],
                    start=(j == 0), stop=(j == jmax))
            if jhi != jmax:
                return
            del live_psr[(h, qi)]
            oc = q0 + 512 * qi
            xin = owork.tile([128, 512], f32, name="xin", tag="xin")
            nc.sync.dma_start(out=xin, in_=xT_d[:, oc : oc + 512].bitcast(f32))
            osb = owork.tile([128, 512], f32, name="osb", tag="osb")
            nc.vector.tensor_add(osb, psr, xin)
            nc.sync.dma_start(out=out_d[:, oc : oc + 512], in_=osb)

        def rcp_group(jlo, jhi):
            """gsum -> 1/(8*gsum) for t-blocks [jlo, jhi]; scale vraw in place."""
            sl = slice(jlo, jhi + 1)
            nc.vector.reduce_sum(
                gsum[:, sl], sums.rearrange("p (j h) -> p j h", h=4)[:, sl, :],
                axis=mybir.AxisListType.X)
            nc.vector.tensor_scalar_mul(gsum[:, sl], gsum[:, sl], float(8.0))
            nc.vector.reciprocal(rcp[:, sl], gsum[:, sl])
            for j in range(jlo, jhi + 1):
                nc.vector.tensor_scalar_mul(
                    vraw[:, V * j : V * (j + 1)], vraw[:, V * j : V * (j + 1)],
                    rcp[:, j : j + 1])

        with tc.tile_pool(name="psB", bufs=2, space="PSUM") as psB, \
             tc.tile_pool(name="psRQ", bufs=4, space="PSUM") as psRQ, \
             tc.tile_pool(name="owork", bufs=2) as owork:
            for half in range(2):
                q0 = HALF * half
                if half == 0:
                    kT_chunk(psRQ, 0)
                def qT_chunk(i):
                    c0, c1 = 512 * i, 512 * (i + 1)
                    psq = psRQ.tile([K, 512], f32, name="psq", tag="psrq")
                    nc.tensor.matmul(psq, wq_sb, xT_sb[:, q0 + c0 : q0 + c1],
                                     start=True, stop=True)
                    nc.vector.tensor_scalar_add(qT_sb[:, c0:c1], psq, bq_sb)

                for i in range(2 if half == 0 else HALF // 512):
                    qT_chunk(i)
                for j in range(_n_t(half)):
                    t0 = 128 * j
                    if half == 0 and j in KT_SCHED:
                        kT_chunk(psRQ, KT_SCHED[j])
                    if half == 0 or j >= 16:
                        # values chunk (bias applied, unnormalized) overlaps exp
                        psv = psRQ.tile([128, V], f32, name="psv0", tag="psrq")
                        nc.tensor.matmul(psv, xT_sb[:, t0 : t0 + 128], wv_sb,
                                         start=True, stop=True)
                        nc.vector.tensor_add(vraw[:, V * j : V * (j + 1)],
                                             psv, bvb_sb)
                    eo, w = e_off[(half, j)]
                    dd = HALF - w  # local q offset of stored region in the half
                    for u, b0 in enumerate(range(0, w, 1024)):
                        if half == 0 and j == 0 and u == 1:
                            qT_chunk(2)
                            qT_chunk(3)
                        bw = min(1024, w - b0)
                        ps = psB.tile([128, 1024], f32, name="ps_l", tag="psB")
                        for s in range(0, bw, 512):
                            sw = min(512, bw - s)
                            nc.tensor.matmul(
                                ps[:, s : s + sw],
                                kT_sb[:, t0 : t0 + 128],
                                qT_sb[:, dd + b0 + s : dd + b0 + s + sw],
                                start=True, stop=True)
                        if b0 == 0 and 128 * j >= q0:  # leading 128 cols = diag
                            nc.vector.tensor_add(ps[:, 0:128], ps[:, 0:128],
                                                 mtri_sb)
                        slot = 4 * j + 2 * half + u
                        if u == 0:
                            nc.scalar.activation(
                                e_all[:, eo + b0 : eo + b0 + bw], ps[:, 0:bw],
                                mybir.ActivationFunctionType.Exp,
                                accum_out=sums[:, slot : slot + 1])
                        else:
                            # 2nd subtile: sum on DVE (slack engine) to keep
                            # the ACT stream free of the accum-read overhead
                            nc.scalar.activation(
                                e_all[:, eo + b0 : eo + b0 + bw], ps[:, 0:bw],
                                mybir.ActivationFunctionType.Exp)
                            nc.vector.reduce_sum(
                                sums[:, slot : slot + 1],
                                e_all[:, eo + b0 : eo + b0 + bw],
                                axis=mybir.AxisListType.X)
                    if half == 1 and j in READ_PLAN:
                        for ev in READ_PLAN[j]:
                            if ev[0] == "rcp":
                                rcp_group(ev[1], ev[2])
                            else:
                                ch, cqi, jlo, jhi = ev
                                read_col_tile(owork, psRQ, ch, cqi, jlo, jhi)

    nc.compile()
    return nc


# Read schedule: column tile (h, qi) depends on t-blocks 0..16h+4qi+3; a
# group g makes rcp for t-blocks [8g, 8g+8) available. Late tiles split
# their reads across groups so most of the read hides under the exp stream.
# Peak live psr accumulators = 4 (psR bufs must cover it).
# kT chunk c is needed by half-0 t-blocks 4c..4c+3 (and all of half 1);
# emit each just before its first consumer so xT DMA arrival never blocks
# the head of the static PE order.
KT_SCHED = {1: 1, 3: 4, 5: 2, 7: 5, 9: 3, 11: 6, 13: 7}

READ_PLAN = {
    7: [("rcp", 0, 7), (0, 0, 0, 3), (0, 1, 0, 7)],
    15: [("rcp", 8, 15), (0, 2, 0, 11), (0, 3, 0, 15)],
    23: [("rcp", 16, 23), (1, 0, 0, 19)],
    24: [(1, 2, 0, 11)],
    25: [(1, 2, 12, 23)],
    27: [("rcp", 24, 27), (1, 1, 0, 11)],
    28: [(1, 1, 12, 23), (1, 2, 24, 27)],
    29: [(1, 3, 0, 13)],
    30: [(1, 3, 14, 27)],
    31: [("rcp", 28, 31), (1, 3, 28, 31)],
}


def _get_nc():
    if "nc" not in _CACHE:
        _CACHE["nc"] = build_program()
    return _CACHE["nc"]


def make_in_maps(inputs):
    mb = np.ascontiguousarray(np.asarray(inputs["minibatch"], dtype=np.float32))
    Wk = np.asarray(inputs["Wk"], dtype=np.float32)
    Wq = np.asarray(inputs["Wq"], dtype=np.float32)
    Wv = np.asarray(inputs["Wv"], dtype=np.float32)
    bk = np.asarray(inputs["bk"], dtype=np.float32).reshape(K, 1)
    bq = np.asarray(inputs["bq"], dtype=np.float32).reshape(K, 1)
    bv = np.asarray(inputs["bv"], dtype=np.float32)

    bvb = np.tile(bv.reshape(1, V), (128, 1)).astype(np.float32)
    p = np.arange(128).reshape(128, 1)
    jj = np.arange(128).reshape(1, 128)
    mtri = np.where(jj >= p, 0.0, NEG).astype(np.float32)

    common = {
        "wkT": np.ascontiguousarray(Wk.T),
        "wqT": np.ascontiguousarray(Wq.T),
        "wvT": np.ascontiguousarray(Wv.T),
        "bk": bk, "bq": bq, "bvb": bvb, "mtri": mtri,
    }
    in_maps = []
    for c in range(NCORES):
        m = dict(common)
        m["xT"] = np.ascontiguousarray(mb[c % B])
        in_maps.append(m)
    return in_maps


def kernel(**inputs):
    from concourse import bass_utils

    nc = _get_nc()
    res = bass_utils.run_bass_kernel_spmd(
        nc, make_in_maps(inputs), core_ids=list(range(NCORES)))
    out = np.stack([res.results[c]["out"] for c in range(B)], axis=0)
    return out.astype(np.float32)


def run_traced(inputs, trace_cores=None):
    """Like kernel() but with NTFF tracing; returns (out, BassKernelResults)."""
    from concourse import bass_utils

    nc = _get_nc()
    res = bass_utils.run_bass_kernel_spmd(
        nc, make_in_maps(inputs), core_ids=list(range(NCORES)),
        trace=True, trace_cores=trace_cores)
    out = np.stack([res.results[c]["out"] for c in range(B)], axis=0)
    return out.astype(np.float32), res
